# revision 37
# baseline (speedup 1.0000x reference)
"""Single-head attention (B=8, S=2048, H=768, D=64) on 8 TRN2 NeuronCores.

Strategy: data-parallel over batch — core b computes batch element b end to
end; no collectives. Host pre-transposes Q/K/V to [H, S] bf16 so every
matmul contraction lands on the partition axis with no device transposes.

Per-core dataflow (all matmuls bf16 x bf16 -> f32 PSUM):
  [qT; kT][128, s] = [Wq|Wk]^T @ [queryT, keyT]  via col-packed matmuls
    (array cols 0-63 run the q projection, 64-127 the k projection,
     concurrently).  v projection packs chunk pairs the same way.
  kT / qT are then partition-duplicated (SBUF->SBUF DMA) into kkT/qqT so
  scores can row-pack: array rows 0-63 compute sk-tile 2j while rows
  64-127 compute sk-tile 2j+1 concurrently (contraction d=64 per group).
  P^T = exp(S^T/8 + mask_bias)   mask bias is per-partition (sk) in this
                                  layout -> fused into the Exp activation.
  O_ext^T[65,sq] = sum_sk V_ext^T @ P^T   (V_ext has a ones column so the
                                  softmax denominator falls out of the AV
                                  matmul as row 64)
  out[sq,d] = PE-transpose(O_ext^T) row-scaled by 1/denom.
"""

import os
from contextlib import ExitStack

import numpy as np
import ml_dtypes

import concourse.bass as bass
import concourse.mybir as mybir
import concourse.tile as tile
from concourse import bacc
from concourse.bass import _add_dep_helper
from concourse.bass_utils import run_bass_kernel_spmd
from concourse.masks import make_identity

S, H, D = 2048, 768, 64
P = 128
NT = S // P      # 16 sk tiles
HT = H // P      # 6 h tiles
CH = 512         # sq chunk for matmul free dim (PSUM bank)
NCH = S // CH    # 4
BF = mybir.dt.bfloat16
F32 = mybir.dt.float32
AF = mybir.ActivationFunctionType

LAST_RESULT = None  # BassKernelResults of the most recent run (for test.py)


def _build(debug=False):
    nc = bacc.Bacc()
    qT_d = nc.declare_dram_parameter("qT", [H, S], BF, isOutput=False)
    kT_d = nc.declare_dram_parameter("kT", [H, S], BF, isOutput=False)
    vT_d = nc.declare_dram_parameter("vT", [H, S], BF, isOutput=False)
    wqk_d = nc.declare_dram_parameter("wqk", [H, P], BF, isOutput=False)
    wvv_d = nc.declare_dram_parameter("wvv", [H, P], BF, isOutput=False)
    bqk_d = nc.declare_dram_parameter("bqk", [P, 1], F32, isOutput=False)
    bvv_d = nc.declare_dram_parameter("bvv", [P, 1], F32, isOutput=False)
    mb_d = nc.declare_dram_parameter("mb", [P, NT], F32, isOutput=False)
    o_d = nc.declare_dram_parameter("o", [S, D], F32, isOutput=True)
    in_d = {"q": qT_d, "k": kT_d, "v": vT_d}

    with ExitStack() as ctx:
        tc = ctx.enter_context(tile.TileContext(nc))
        consts = ctx.enter_context(tc.tile_pool(name="consts", bufs=1))
        stage = ctx.enter_context(tc.tile_pool(name="stage", bufs=6 * HT))
        persist = ctx.enter_context(tc.tile_pool(name="persist", bufs=1))
        ppool = ctx.enter_context(tc.tile_pool(name="ppool", bufs=20))
        ostage = ctx.enter_context(tc.tile_pool(name="ostage", bufs=3))
        small = ctx.enter_context(tc.tile_pool(name="small", bufs=3))
        psw = ctx.enter_context(tc.tile_pool(name="psw", bufs=2, space="PSUM"))
        pso = ctx.enter_context(tc.tile_pool(name="pso", bufs=1, space="PSUM"))

        # ---- weights + input load first so transfers start immediately;
        # every tile lands in a fresh slot (no recycling -> no HWDGE waits).
        # Inputs stream as [128, S/2] halves in the order the projection
        # chunks consume them, so the first matmul starts ~4us in.
        w_sb = consts.tile([P, 2, HT, P], BF, tag="w")  # wqk | wvv h-tiles
        nc.sync.dma_start(
            out=w_sb[:, 0, :, :],
            in_=wqk_d[:, :].rearrange("(t p) n -> p t n", p=P),
        )
        nc.sync.dma_start(
            out=w_sb[:, 1, :, :],
            in_=wvv_d[:, :].rearrange("(t p) n -> p t n", p=P),
        )
        HS = S // 2
        st_all = {}
        # single sync queue, issued in consumption order: q/k half0,
        # q/k half1, then v — so the critical q/k bytes never contend
        # with v for HBM bandwidth
        for half in range(2):
            for t in "qk":
                for h in range(HT):
                    st = stage.tile(
                        [P, HS], BF, tag="stage", name=f"st_{t}{h}{half}"
                    )
                    nc.sync.dma_start(
                        out=st,
                        in_=in_d[t][
                            h * P : (h + 1) * P, half * HS : (half + 1) * HS
                        ],
                    )
                    st_all[t, h, half] = st
        for half in range(2):
            for h in range(HT):
                st = stage.tile([P, HS], BF, tag="stage", name=f"st_v{h}{half}")
                nc.sync.dma_start(
                    out=st,
                    in_=vT_d[h * P : (h + 1) * P, half * HS : (half + 1) * HS],
                )
                st_all["v", h, half] = st

        # ---- other constants ----
        bqk_sb = consts.tile([P, 1], F32, tag="bqk")
        nc.sync.dma_start(out=bqk_sb, in_=bqk_d[:, :])
        bvv_sb = consts.tile([P, 1], F32, tag="bvv")
        nc.sync.dma_start(out=bvv_sb, in_=bvv_d[:, :])
        mb_sb = consts.tile([P, NT], F32, tag="mb")
        nc.sync.dma_start(out=mb_sb, in_=mb_d[:, :])
        ident = consts.tile([P, P], F32, tag="ident")
        make_identity(nc, ident)
        ident_bf = consts.tile([P, P], BF, tag="ident_bf")
        make_identity(nc, ident_bf)

        # ---- persistent SBUF tensors ----
        qqT_sb = persist.tile([P, S], BF, tag="qqT")  # qT in both halves
        kkT_sb = persist.tile([P, S], BF, tag="kkT")  # kT in both halves
        vT2_sb = persist.tile([P, S // 2], BF, tag="vT2")  # vT chunk pairs
        vE_sb = persist.tile([P, NT * (D + 1)], BF, tag="vE")  # V_ext tiles
        nc.vector.memset(vE_sb, 1.0)  # ones column (col 64 of each tile)
        oT_sb = persist.tile([D + 1, S], F32, tag="oT")  # O_ext^T staging

        # helper blocks -------------------------------------------------
        def qk_proj_chunk(c):
            """col-packed q/k projection for sq chunk c, then the PE
            identity-matmul partition duplication for that chunk."""
            hf, cc = c // 2, c % 2
            pp = pso.tile([P, CH], F32, tag=f"o{c}", name=f"pp{c}")
            for h in range(HT):
                nc.tensor.matmul(
                    pp[:D, :],
                    lhsT=w_sb[:, 0, h, :D],
                    rhs=st_all["q", h, hf][:, cc * CH : (cc + 1) * CH],
                    start=(h == 0),
                    stop=(h == HT - 1),
                    tile_position=(0, 0),
                    skip_group_check=True,
                )
                nc.tensor.matmul(
                    pp[D:, :],
                    lhsT=w_sb[:, 0, h, D:],
                    rhs=st_all["k", h, hf][:, cc * CH : (cc + 1) * CH],
                    start=(h == 0),
                    stop=(h == HT - 1),
                    tile_position=(0, D),
                    skip_group_check=True,
                )
            ch = slice(c * CH, (c + 1) * CH)
            nc.vector.tensor_scalar_add(
                out=qqT_sb[:D, ch], in0=pp[:D, :], scalar1=bqk_sb[:D, :]
            )
            nc.vector.tensor_scalar_add(
                out=kkT_sb[D:, ch], in0=pp[D:, :], scalar1=bqk_sb[D:, :]
            )
            pd = psw.tile([P, CH], F32, tag="work", name=f"pd{c}")
            nc.tensor.matmul(
                pd[D:, :],
                lhsT=ident_bf[:D, :D],
                rhs=qqT_sb[:D, ch],
                start=True,
                stop=True,
                tile_position=(0, D),
                skip_group_check=True,
            )
            nc.tensor.matmul(
                pd[:D, :],
                lhsT=ident_bf[D:, D:],
                rhs=kkT_sb[D:, ch],
                start=True,
                stop=True,
                tile_position=(D, 0),
                skip_group_check=True,
            )
            nc.vector.tensor_copy(out=qqT_sb[D:, ch], in_=pd[D:, :])
            nc.vector.tensor_copy(out=kkT_sb[:D, ch], in_=pd[:D, :])

        pth = {}

        def scores_exp(j, half):
            """row-packed scores for sk-tile pair (2j, 2j+1) over sq half,
            exp straight into bf16 half-tiles."""
            ta, tb = 2 * j, 2 * j + 1
            ps_a = psw.tile([P, 2 * CH], F32, tag="work", name=f"psa{j}{half}")
            ps_b = psw.tile([P, 2 * CH], F32, tag="work", name=f"psb{j}{half}")
            for sub in range(2):
                c = 2 * half + sub
                nc.tensor.matmul(
                    ps_a[:, sub * CH : (sub + 1) * CH],
                    lhsT=kkT_sb[:D, ta * P : (ta + 1) * P],
                    rhs=qqT_sb[:D, c * CH : (c + 1) * CH],
                    start=True,
                    stop=True,
                    tile_position=(0, 0),
                )
                nc.tensor.matmul(
                    ps_b[:, sub * CH : (sub + 1) * CH],
                    lhsT=kkT_sb[D:, tb * P : (tb + 1) * P],
                    rhs=qqT_sb[D:, c * CH : (c + 1) * CH],
                    start=True,
                    stop=True,
                    tile_position=(D, 0),
                )
            for t, ps in ((ta, ps_a), (tb, ps_b)):
                pt = ppool.tile(
                    [P, 2 * CH], BF, tag="pT", name=f"pt{t}_{half}"
                )
                nc.scalar.activation(
                    out=pt,
                    in_=ps,
                    func=AF.Exp,
                    bias=mb_sb[:, t : t + 1],
                    scale=0.125,
                )
                pth[t, half] = pt

        po = [
            pso.tile([D + 1, CH], F32, tag=f"o{c}", name=f"po{c}")
            for c in range(NCH)
        ]

        def av(t, cs):
            for c in cs:
                nc.tensor.matmul(
                    po[c],
                    lhsT=vE_sb[:, t * (D + 1) : (t + 1) * (D + 1)],
                    rhs=pth[t, c // 2][:, (c % 2) * CH : (c % 2 + 1) * CH],
                    start=(t == 0),
                    stop=(t == NT - 1),
                )

        # interleaved schedule: the exp chain (the ACT-bound floor) starts
        # as soon as qq/kk chunks 0-1 exist, and everything else hides
        # under it -------------------------------------------------------
        qk_proj_chunk(0)
        qk_proj_chunk(1)
        scores_exp(0, 0)
        scores_exp(1, 0)
        qk_proj_chunk(2)
        qk_proj_chunk(3)
        for j in range(2, NT // 2):
            scores_exp(j, 0)

        # ---- v projection, chunk pairs packed: rows 0:64 <- chunk 2u,
        # rows 64:128 <- chunk 2u+1; then PE-transpose [64, 128] pieces
        # into V_ext [128, 64] tiles ----
        for u in range(NCH // 2):
            pv = pso.tile([P, CH], F32, tag=f"o{2 * u}", name=f"pv{u}")
            for h in range(HT):
                nc.tensor.matmul(
                    pv[:D, :],
                    lhsT=w_sb[:, 1, h, :D],
                    rhs=st_all["v", h, u][:, :CH],
                    start=(h == 0),
                    stop=(h == HT - 1),
                    tile_position=(0, 0),
                    skip_group_check=True,
                )
                nc.tensor.matmul(
                    pv[D:, :],
                    lhsT=w_sb[:, 1, h, D:],
                    rhs=st_all["v", h, u][:, CH:],
                    start=(h == 0),
                    stop=(h == HT - 1),
                    tile_position=(0, D),
                    skip_group_check=True,
                )
            nc.vector.tensor_scalar_add(
                out=vT2_sb[:, u * CH : (u + 1) * CH], in0=pv, scalar1=bvv_sb
            )
        for k in range(NT):
            c = k // 4  # original chunk index
            base = (c // 2) * CH + (k % 4) * P
            lo, hi = (0, D) if c % 2 == 0 else (D, P)
            pt = psw.tile([P, D], BF, tag="work", name=f"ptv{k}")
            nc.tensor.transpose(
                pt,
                in_=vT2_sb[lo:hi, base : base + P],
                identity=ident_bf[lo:hi, lo:hi],
            )
            nc.vector.tensor_copy(
                out=vE_sb[:, k * (D + 1) : k * (D + 1) + D], in_=pt
            )

        # second sq half + AV, pairwise so pT half-tiles recycle promptly
        for j in range(NT // 2):
            scores_exp(j, 1)
            av(2 * j, range(NCH))
            av(2 * j + 1, range(NCH))

        # ---- epilogue: normalize + transpose to [sq, d] ----
        for c in range(NCH):
            nc.vector.tensor_copy(out=oT_sb[:, c * CH : (c + 1) * CH], in_=po[c])
        for k in range(NT):
            pt = psw.tile([P, D + 1], F32, tag="work", name=f"pto{k}")
            nc.tensor.transpose(
                pt,
                in_=oT_sb[:, k * P : (k + 1) * P],
                identity=ident[: D + 1, : D + 1],
            )
            r = small.tile([P, 1], F32, tag="recip", name=f"r{k}")
            nc.vector.reciprocal(r, pt[:, D : D + 1])
            ot = ostage.tile([P, D], F32, tag="ot", name=f"ot{k}")
            nc.vector.tensor_scalar_mul(ot, pt[:, :D], r)
            nc.sync.dma_start(out=o_d[k * P : (k + 1) * P, :], in_=ot)

        if debug:
            for nm, t in [
                ("dbg_qq", qqT_sb),
                ("dbg_kk", kkT_sb),
                ("dbg_vE", vE_sb),
                ("dbg_vT2", vT2_sb),
            ]:
                dd = nc.declare_dram_parameter(nm, list(t.shape), BF, isOutput=True)
                nc.sync.dma_start(out=dd[:, :], in_=t)
            do = nc.declare_dram_parameter(
                "dbg_oT", [D + 1, S], F32, isOutput=True
            )
            nc.sync.dma_start(out=do[:, :], in_=oT_sb)

    return nc


_NC = None


def kernel(query, key, value, mask, Wq, bq, Wk, bk, Wv, bv):
    global _NC, LAST_RESULT
    bf16 = ml_dtypes.bfloat16
    B = query.shape[0]
    assert B == 8

    if _NC is None:
        _NC = _build()
        _NC.finalize()  # run bacc passes (wait splitting, reg alloc, ACT tables)

    wqk = np.ascontiguousarray(
        np.concatenate([np.asarray(Wq), np.asarray(Wk)], axis=1).astype(bf16)
    )
    wvv = np.ascontiguousarray(
        np.concatenate([np.asarray(Wv), np.asarray(Wv)], axis=1).astype(bf16)
    )
    bqk = np.concatenate([np.asarray(bq), np.asarray(bk)]).astype(np.float32)
    bvv = np.concatenate([np.asarray(bv), np.asarray(bv)]).astype(np.float32)

    in_maps = []
    for b in range(B):
        mb = ((np.asarray(mask[b], np.float32) - 1.0) * 1e9).reshape(NT, P).T
        in_maps.append(
            {
                "qT": np.ascontiguousarray(np.asarray(query[b]).T.astype(bf16)),
                "kT": np.ascontiguousarray(np.asarray(key[b]).T.astype(bf16)),
                "vT": np.ascontiguousarray(np.asarray(value[b]).T.astype(bf16)),
                "wqk": wqk,
                "wvv": wvv,
                "bqk": bqk.reshape(P, 1),
                "bvv": bvv.reshape(P, 1),
                "mb": np.ascontiguousarray(mb),
            }
        )

    res = run_bass_kernel_spmd(
        _NC,
        in_maps,
        core_ids=list(range(8)),
        trace=bool(os.environ.get("KERNEL_TRACE")),
    )
    LAST_RESULT = res
    out = np.stack([np.asarray(res.results[i]["o"]) for i in range(B)])
    return out.astype(np.float32)


# revision 38
# speedup vs baseline: 1.0456x; 1.0456x over previous
"""Single-head attention (B=8, S=2048, H=768, D=64) on 8 TRN2 NeuronCores.

Strategy: data-parallel over batch — core b computes batch element b end to
end; no collectives. Host pre-transposes Q/K/V to [H, S] bf16 so every
matmul contraction lands on the partition axis with no device transposes.

Per-core dataflow (all matmuls bf16 x bf16 -> f32 PSUM):
  [qT; kT][128, s] = [Wq|Wk]^T @ [queryT, keyT]  via col-packed matmuls
    (array cols 0-63 run the q projection, 64-127 the k projection,
     concurrently).  v projection packs chunk pairs the same way.
  kT / qT are then partition-duplicated (SBUF->SBUF DMA) into kkT/qqT so
  scores can row-pack: array rows 0-63 compute sk-tile 2j while rows
  64-127 compute sk-tile 2j+1 concurrently (contraction d=64 per group).
  P^T = exp(S^T/8 + mask_bias)   mask bias is per-partition (sk) in this
                                  layout -> fused into the Exp activation.
  O_ext^T[65,sq] = sum_sk V_ext^T @ P^T   (V_ext has a ones column so the
                                  softmax denominator falls out of the AV
                                  matmul as row 64)
  out[sq,d] = PE-transpose(O_ext^T) row-scaled by 1/denom.
"""

import os
from contextlib import ExitStack

import numpy as np
import ml_dtypes

import concourse.bass as bass
import concourse.mybir as mybir
import concourse.tile as tile
from concourse import bacc
from concourse.bass import _add_dep_helper
from concourse.bass_utils import run_bass_kernel_spmd
from concourse.masks import make_identity

S, H, D = 2048, 768, 64
P = 128
NT = S // P      # 16 sk tiles
HT = H // P      # 6 h tiles
CH = 512         # sq chunk for matmul free dim (PSUM bank)
NCH = S // CH    # 4
BF = mybir.dt.bfloat16
F32 = mybir.dt.float32
AF = mybir.ActivationFunctionType

LAST_RESULT = None  # BassKernelResults of the most recent run (for test.py)


def _build(debug=False):
    nc = bacc.Bacc()
    qT_d = nc.declare_dram_parameter("qT", [H, S], BF, isOutput=False)
    kT_d = nc.declare_dram_parameter("kT", [H, S], BF, isOutput=False)
    vT_d = nc.declare_dram_parameter("vT", [H, S], BF, isOutput=False)
    wqk_d = nc.declare_dram_parameter("wqk", [H, P], BF, isOutput=False)
    wvv_d = nc.declare_dram_parameter("wvv", [H, P], BF, isOutput=False)
    bqk_d = nc.declare_dram_parameter("bqk", [P, 1], F32, isOutput=False)
    bvv_d = nc.declare_dram_parameter("bvv", [P, 1], F32, isOutput=False)
    mb_d = nc.declare_dram_parameter("mb", [P, NT], F32, isOutput=False)
    o_d = nc.declare_dram_parameter("o", [S, D], F32, isOutput=True)
    in_d = {"q": qT_d, "k": kT_d, "v": vT_d}

    with ExitStack() as ctx:
        tc = ctx.enter_context(tile.TileContext(nc))
        consts = ctx.enter_context(tc.tile_pool(name="consts", bufs=1))
        stage = ctx.enter_context(tc.tile_pool(name="stage", bufs=6 * HT))
        persist = ctx.enter_context(tc.tile_pool(name="persist", bufs=1))
        ppool = ctx.enter_context(tc.tile_pool(name="ppool", bufs=20))
        ostage = ctx.enter_context(tc.tile_pool(name="ostage", bufs=3))
        small = ctx.enter_context(tc.tile_pool(name="small", bufs=3))
        psw = ctx.enter_context(tc.tile_pool(name="psw", bufs=2, space="PSUM"))
        pso = ctx.enter_context(tc.tile_pool(name="pso", bufs=1, space="PSUM"))

        # ---- weights + input load first so transfers start immediately;
        # every tile lands in a fresh slot (no recycling -> no HWDGE waits).
        # Inputs stream as [128, S/2] halves in the order the projection
        # chunks consume them, so the first matmul starts ~4us in.
        w_sb = consts.tile([P, 2, HT, P], BF, tag="w")  # wqk | wvv h-tiles
        nc.sync.dma_start(
            out=w_sb[:, 0, :, :],
            in_=wqk_d[:, :].rearrange("(t p) n -> p t n", p=P),
        )
        nc.sync.dma_start(
            out=w_sb[:, 1, :, :],
            in_=wvv_d[:, :].rearrange("(t p) n -> p t n", p=P),
        )
        HS = S // 2
        st_all = {}
        # single sync queue, issued in consumption order: q/k half0,
        # q/k half1, then v — so the critical q/k bytes never contend
        # with v for HBM bandwidth
        for half in range(2):
            for t in "qk":
                for h in range(HT):
                    st = stage.tile(
                        [P, HS], BF, tag="stage", name=f"st_{t}{h}{half}"
                    )
                    nc.sync.dma_start(
                        out=st,
                        in_=in_d[t][
                            h * P : (h + 1) * P, half * HS : (half + 1) * HS
                        ],
                    )
                    st_all[t, h, half] = st
        # v issues in parallel on the Activation HWDGE queue (idle until the
        # first exp ~40us later), so it lands well before the v projection
        for half in range(2):
            for h in range(HT):
                st = stage.tile([P, HS], BF, tag="stage", name=f"st_v{h}{half}")
                nc.scalar.dma_start(
                    out=st,
                    in_=vT_d[h * P : (h + 1) * P, half * HS : (half + 1) * HS],
                )
                st_all["v", h, half] = st

        # ---- other constants ----
        bqk_sb = consts.tile([P, 1], F32, tag="bqk")
        nc.sync.dma_start(out=bqk_sb, in_=bqk_d[:, :])
        bvv_sb = consts.tile([P, 1], F32, tag="bvv")
        nc.sync.dma_start(out=bvv_sb, in_=bvv_d[:, :])
        mb_sb = consts.tile([P, NT], F32, tag="mb")
        nc.sync.dma_start(out=mb_sb, in_=mb_d[:, :])
        ident = consts.tile([P, P], F32, tag="ident")
        make_identity(nc, ident)
        ident_bf = consts.tile([P, P], BF, tag="ident_bf")
        make_identity(nc, ident_bf)

        # ---- persistent SBUF tensors ----
        qqT_sb = persist.tile([P, S], BF, tag="qqT")  # qT in both halves
        kkT_sb = persist.tile([P, S], BF, tag="kkT")  # kT in both halves
        vT2_sb = persist.tile([P, S // 2], BF, tag="vT2")  # vT chunk pairs
        vE_sb = persist.tile([P, NT * (D + 1)], BF, tag="vE")  # V_ext tiles
        nc.vector.memset(vE_sb, 1.0)  # ones column (col 64 of each tile)
        oT_sb = persist.tile([D + 1, S], F32, tag="oT")  # O_ext^T staging

        # helper blocks -------------------------------------------------
        def qk_proj_chunk(c):
            """col-packed q/k projection for sq chunk c, then the PE
            identity-matmul partition duplication for that chunk."""
            hf, cc = c // 2, c % 2
            pp = pso.tile([P, CH], F32, tag=f"o{c}", name=f"pp{c}")
            for h in range(HT):
                nc.tensor.matmul(
                    pp[:D, :],
                    lhsT=w_sb[:, 0, h, :D],
                    rhs=st_all["q", h, hf][:, cc * CH : (cc + 1) * CH],
                    start=(h == 0),
                    stop=(h == HT - 1),
                    tile_position=(0, 0),
                    skip_group_check=True,
                )
                nc.tensor.matmul(
                    pp[D:, :],
                    lhsT=w_sb[:, 0, h, D:],
                    rhs=st_all["k", h, hf][:, cc * CH : (cc + 1) * CH],
                    start=(h == 0),
                    stop=(h == HT - 1),
                    tile_position=(0, D),
                    skip_group_check=True,
                )
            ch = slice(c * CH, (c + 1) * CH)
            nc.vector.tensor_scalar_add(
                out=qqT_sb[:D, ch], in0=pp[:D, :], scalar1=bqk_sb[:D, :]
            )
            nc.vector.tensor_scalar_add(
                out=kkT_sb[D:, ch], in0=pp[D:, :], scalar1=bqk_sb[D:, :]
            )
            pd = psw.tile([P, CH], F32, tag="work", name=f"pd{c}")
            nc.tensor.matmul(
                pd[D:, :],
                lhsT=ident_bf[:D, :D],
                rhs=qqT_sb[:D, ch],
                start=True,
                stop=True,
                tile_position=(0, D),
                skip_group_check=True,
            )
            nc.tensor.matmul(
                pd[:D, :],
                lhsT=ident_bf[D:, D:],
                rhs=kkT_sb[D:, ch],
                start=True,
                stop=True,
                tile_position=(D, 0),
                skip_group_check=True,
            )
            nc.vector.tensor_copy(out=qqT_sb[D:, ch], in_=pd[D:, :])
            nc.vector.tensor_copy(out=kkT_sb[:D, ch], in_=pd[:D, :])

        pth = {}

        def scores_exp(j, half):
            """row-packed scores for sk-tile pair (2j, 2j+1) over sq half,
            exp straight into bf16 half-tiles."""
            ta, tb = 2 * j, 2 * j + 1
            ps_a = psw.tile([P, 2 * CH], F32, tag="work", name=f"psa{j}{half}")
            ps_b = psw.tile([P, 2 * CH], F32, tag="work", name=f"psb{j}{half}")
            for sub in range(2):
                c = 2 * half + sub
                nc.tensor.matmul(
                    ps_a[:, sub * CH : (sub + 1) * CH],
                    lhsT=kkT_sb[:D, ta * P : (ta + 1) * P],
                    rhs=qqT_sb[:D, c * CH : (c + 1) * CH],
                    start=True,
                    stop=True,
                    tile_position=(0, 0),
                )
                nc.tensor.matmul(
                    ps_b[:, sub * CH : (sub + 1) * CH],
                    lhsT=kkT_sb[D:, tb * P : (tb + 1) * P],
                    rhs=qqT_sb[D:, c * CH : (c + 1) * CH],
                    start=True,
                    stop=True,
                    tile_position=(D, 0),
                )
            for t, ps in ((ta, ps_a), (tb, ps_b)):
                pt = ppool.tile(
                    [P, 2 * CH], BF, tag="pT", name=f"pt{t}_{half}"
                )
                nc.scalar.activation(
                    out=pt,
                    in_=ps,
                    func=AF.Exp,
                    bias=mb_sb[:, t : t + 1],
                    scale=0.125,
                )
                pth[t, half] = pt

        po = [
            pso.tile([D + 1, CH], F32, tag=f"o{c}", name=f"po{c}")
            for c in range(NCH)
        ]

        def av(t, cs):
            for c in cs:
                nc.tensor.matmul(
                    po[c],
                    lhsT=vE_sb[:, t * (D + 1) : (t + 1) * (D + 1)],
                    rhs=pth[t, c // 2][:, (c % 2) * CH : (c % 2 + 1) * CH],
                    start=(t == 0),
                    stop=(t == NT - 1),
                )

        # interleaved schedule: the exp chain (the ACT-bound floor) starts
        # as soon as qq/kk chunks 0-1 exist, and everything else hides
        # under it -------------------------------------------------------
        qk_proj_chunk(0)
        qk_proj_chunk(1)
        scores_exp(0, 0)
        scores_exp(1, 0)
        qk_proj_chunk(2)
        qk_proj_chunk(3)
        for j in range(2, NT // 2):
            scores_exp(j, 0)

        # ---- v projection, chunk pairs packed: rows 0:64 <- chunk 2u,
        # rows 64:128 <- chunk 2u+1; then PE-transpose [64, 128] pieces
        # into V_ext [128, 64] tiles ----
        for u in range(NCH // 2):
            pv = pso.tile([P, CH], F32, tag=f"o{2 * u}", name=f"pv{u}")
            for h in range(HT):
                nc.tensor.matmul(
                    pv[:D, :],
                    lhsT=w_sb[:, 1, h, :D],
                    rhs=st_all["v", h, u][:, :CH],
                    start=(h == 0),
                    stop=(h == HT - 1),
                    tile_position=(0, 0),
                    skip_group_check=True,
                )
                nc.tensor.matmul(
                    pv[D:, :],
                    lhsT=w_sb[:, 1, h, D:],
                    rhs=st_all["v", h, u][:, CH:],
                    start=(h == 0),
                    stop=(h == HT - 1),
                    tile_position=(0, D),
                    skip_group_check=True,
                )
            nc.vector.tensor_scalar_add(
                out=vT2_sb[:, u * CH : (u + 1) * CH], in0=pv, scalar1=bvv_sb
            )
        for k in range(NT):
            c = k // 4  # original chunk index
            base = (c // 2) * CH + (k % 4) * P
            lo, hi = (0, D) if c % 2 == 0 else (D, P)
            pt = psw.tile([P, D], BF, tag="work", name=f"ptv{k}")
            nc.tensor.transpose(
                pt,
                in_=vT2_sb[lo:hi, base : base + P],
                identity=ident_bf[lo:hi, lo:hi],
            )
            nc.vector.tensor_copy(
                out=vE_sb[:, k * (D + 1) : k * (D + 1) + D], in_=pt
            )

        # second sq half + AV, pairwise so pT half-tiles recycle promptly
        for j in range(NT // 2):
            scores_exp(j, 1)
            av(2 * j, range(NCH))
            av(2 * j + 1, range(NCH))

        # ---- epilogue: normalize + transpose to [sq, d] ----
        for c in range(NCH):
            nc.vector.tensor_copy(out=oT_sb[:, c * CH : (c + 1) * CH], in_=po[c])
        for k in range(NT):
            pt = psw.tile([P, D + 1], F32, tag="work", name=f"pto{k}")
            nc.tensor.transpose(
                pt,
                in_=oT_sb[:, k * P : (k + 1) * P],
                identity=ident[: D + 1, : D + 1],
            )
            r = small.tile([P, 1], F32, tag="recip", name=f"r{k}")
            nc.vector.reciprocal(r, pt[:, D : D + 1])
            ot = ostage.tile([P, D], F32, tag="ot", name=f"ot{k}")
            nc.vector.tensor_scalar_mul(ot, pt[:, :D], r)
            nc.sync.dma_start(out=o_d[k * P : (k + 1) * P, :], in_=ot)

        if debug:
            for nm, t in [
                ("dbg_qq", qqT_sb),
                ("dbg_kk", kkT_sb),
                ("dbg_vE", vE_sb),
                ("dbg_vT2", vT2_sb),
            ]:
                dd = nc.declare_dram_parameter(nm, list(t.shape), BF, isOutput=True)
                nc.sync.dma_start(out=dd[:, :], in_=t)
            do = nc.declare_dram_parameter(
                "dbg_oT", [D + 1, S], F32, isOutput=True
            )
            nc.sync.dma_start(out=do[:, :], in_=oT_sb)

    return nc


_NC = None


def kernel(query, key, value, mask, Wq, bq, Wk, bk, Wv, bv):
    global _NC, LAST_RESULT
    bf16 = ml_dtypes.bfloat16
    B = query.shape[0]
    assert B == 8

    if _NC is None:
        _NC = _build()
        _NC.finalize()  # run bacc passes (wait splitting, reg alloc, ACT tables)

    wqk = np.ascontiguousarray(
        np.concatenate([np.asarray(Wq), np.asarray(Wk)], axis=1).astype(bf16)
    )
    wvv = np.ascontiguousarray(
        np.concatenate([np.asarray(Wv), np.asarray(Wv)], axis=1).astype(bf16)
    )
    bqk = np.concatenate([np.asarray(bq), np.asarray(bk)]).astype(np.float32)
    bvv = np.concatenate([np.asarray(bv), np.asarray(bv)]).astype(np.float32)

    in_maps = []
    for b in range(B):
        mb = ((np.asarray(mask[b], np.float32) - 1.0) * 1e9).reshape(NT, P).T
        in_maps.append(
            {
                "qT": np.ascontiguousarray(np.asarray(query[b]).T.astype(bf16)),
                "kT": np.ascontiguousarray(np.asarray(key[b]).T.astype(bf16)),
                "vT": np.ascontiguousarray(np.asarray(value[b]).T.astype(bf16)),
                "wqk": wqk,
                "wvv": wvv,
                "bqk": bqk.reshape(P, 1),
                "bvv": bvv.reshape(P, 1),
                "mb": np.ascontiguousarray(mb),
            }
        )

    res = run_bass_kernel_spmd(
        _NC,
        in_maps,
        core_ids=list(range(8)),
        trace=bool(os.environ.get("KERNEL_TRACE")),
    )
    LAST_RESULT = res
    out = np.stack([np.asarray(res.results[i]["o"]) for i in range(B)])
    return out.astype(np.float32)
